# revision 29
# baseline (speedup 1.0000x reference)
"""Block-sparse attention Trainium2 kernel (8 NeuronCores, SPMD).

Problem: hidden_states [2, 2048, 2048] fp32; Wq/Wk/Wv [2048, 2048]; Wo
[2048, 2048]. 16 heads x 128 dim, block-banded attention (BLOCK=64,
bandwidth 2 -> each 128-query tile attends a 384-key band with two
64x64 invalid corners).

Sharding: core c = (batch b = c//4) x (head group g = c%4, 4 heads).
Each core computes q/k/v projections for its 4 heads (columns of
Wq/Wk/Wv), banded attention, and a partial output through its rows of
Wo. Host sums the 4 partials per batch. No collectives.

Per-core schedule (all matmuls bf16, fp32 PSUM accumulate; inputs
pre-transposed/cast to bf16 host-side during sharding):
  1. A zeroed warm-up tile is memset first so ~32 dependency-free
     matmuls ramp the PE DVFS clock before any DMA data arrives;
     the transpose identity is built after them.
  2. Loads are k-striped across the three DMA queues (sync/scalar/
     gpsimd) in consumption order: (wq[k], hT[k] first half) pairs,
     then (wk[k], hT[k] second half), then wv/wo.
  3. Q/K projections run as k-synchronized 8-chain WAVES holding all
     8 PSUM banks (a scoped pool): every arriving (w[k], hT[k]) pair
     feeds 8*512 PE cycles, keeping the PE dense and its DVFS clock
     at the K=8 (2.4GHz) state.  QT/KT are produced directly
     transposed (lhsT=weight slice, rhs=hT) and persist for all 4
     heads.
  4. One interleaved slot loop runs everything else: slot t does the
     V-projection tile V_t, attention tiles att_h(t-1-h) for the four
     heads, and the output-projection group Wo(t-6).  Attention lags
     satisfy the V band (qt+1 <= t) and every Wo group's AO inputs
     finish two slots earlier, so the PE never waits on an in-flight
     softmax chain and no phase goes sparse (which would demote the
     PE clock).
  Attention tile: scores = QT^T KT band -> corner mask via two 64x64
  PSUM memsets (vector engine; no mask tiles, no mask-add pass) ->
  exp straight out of PSUM with fused rowsum accumulation (no max
  subtract; scores are O(+-8)) -> reciprocal -> normalize P -> PE-
  transpose P chunks grouped so the identity stationary loads once ->
  PV accumulation -> AO^T bf16.
  Wo group: out[mt] = sum_h AO_h^T^T @ Wo rows, PSUM->SBUF copies
  alternating vector/scalar, output DMAs on the sync queue (the last
  groups split each copy AND its DMA across both engines/queues).
"""

from contextlib import ExitStack

import numpy as np

import concourse.bass as bass
import concourse.mybir as mybir
import concourse.tile as tile
from concourse import bacc
from concourse.bass_utils import run_bass_kernel_spmd
from concourse.masks import make_identity

S = 2048          # sequence length
HID = 2048        # hidden size
HL = 4            # heads per core
D = 128           # head dim
NKT = HID // 128  # 16 contraction tiles
NQ = S // 128     # 16 query tiles
SCALE = float(D) ** -0.5
NEG = -1e30
BF = mybir.dt.bfloat16
F32 = mybir.dt.float32


def _emit_wo(nc, ps_wo, osb_pool, AO_T, wo_s, out, mt, out_engs=None, fine=False):
    mts = slice(128 * mt, 128 * (mt + 1))
    if out_engs is None:
        out_engs = (nc.sync,)
    for nc_ in range(4):
        ns = slice(512 * nc_, 512 * (nc_ + 1))
        ops_ = ps_wo.tile([128, 512], mybir.dt.float32, tag="wo", name="wops")
        for dk in range(HL):
            nc.tensor.matmul(
                ops_, lhsT=AO_T[dk][:, mts], rhs=wo_s[dk][:, ns],
                start=(dk == 0), stop=(dk == HL - 1),
            )
        osb = osb_pool.tile([128, 512], BF, tag="osb", name="osb")
        if fine:
            # tail: both engines drain half the tile in parallel, and
            # each half DMAs out on its own queue as soon as it lands
            nc.vector.tensor_copy(osb[:, 0:256], ops_[:, 0:256])
            nc.scalar.copy(osb[:, 256:512], ops_[:, 256:512])
            ns0 = slice(512 * nc_, 512 * nc_ + 256)
            ns1 = slice(512 * nc_ + 256, 512 * (nc_ + 1))
            e0 = out_engs[nc_ % len(out_engs)]
            e1 = out_engs[(nc_ + 1) % len(out_engs)]
            e0.dma_start(out=out[mts, ns0], in_=osb[:, 0:256])
            e1.dma_start(out=out[mts, ns1], in_=osb[:, 256:512])
        else:
            if nc_ % 2 == 0:
                nc.vector.tensor_copy(osb, ops_)
            else:
                nc.scalar.copy(osb, ops_)
            out_engs[nc_ % len(out_engs)].dma_start(out=out[mts, ns], in_=osb)


def build():
    nc = bacc.Bacc()
    # ht = h^T [hidden, seq]; all inputs pre-transposed/cast to bf16
    # host-side during sharding
    ht = nc.declare_dram_parameter("ht", [HID, S], BF, isOutput=False)
    wq = nc.declare_dram_parameter("wq", [HID, HL * D], BF, isOutput=False)
    wk = nc.declare_dram_parameter("wk", [HID, HL * D], BF, isOutput=False)
    wv = nc.declare_dram_parameter("wv", [HID, HL * D], BF, isOutput=False)
    wo = nc.declare_dram_parameter("wo", [HL * D, HID], BF, isOutput=False)
    out = nc.declare_dram_parameter("out", [S, HID], BF, isOutput=True)

    with ExitStack() as ctx:
        tc = ctx.enter_context(tile.TileContext(nc))
        persist = ctx.enter_context(tc.tile_pool(name="persist", bufs=1))
        work = ctx.enter_context(tc.tile_pool(name="work", bufs=4))
        stats = ctx.enter_context(tc.tile_pool(name="stats", bufs=8))
        osb_pool = ctx.enter_context(tc.tile_pool(name="osb", bufs=4))
        # the Q/K projection waves get ALL 8 PSUM banks via a scoped
        # pool; the attention/V/Wo pools are allocated after it closes
        wave_ctx = ExitStack()
        ps_wave = wave_ctx.enter_context(
            tc.tile_pool(name="ps_wave", bufs=8, space="PSUM"))

        # warm-up stationary: a zeroed tile whose memset is the very
        # first gpsimd op, so the PE ramps before any DMA data arrives
        warm_sb = persist.tile([128, 128], BF, tag="warm_sb")
        nc.gpsimd.memset(warm_sb, 0.0)
        warm_ps = ps_wave.tile([128, 512], F32, tag="big", name="warm_ps")
        for _ in range(56):
            nc.tensor.matmul(warm_ps[:, 0:128], lhsT=warm_sb, rhs=warm_sb,
                             start=True, stop=True)

        ident = persist.tile([128, 128], BF, tag="ident")
        make_identity(nc, ident)

        # ---- input loads (plain 2D DMAs, bf16), k-striped over the
        # three queues so arrival tracks the k-synchronized consumption
        # order; within each queue strictly priority-ordered.
        hT = [persist.tile([128, S], BF, tag=f"ht{k}", name=f"ht{k}") for k in range(NKT)]
        wq_s = [persist.tile([128, HL * D], BF, tag=f"wq{k}", name=f"wq{k}") for k in range(NKT)]
        wk_s = [persist.tile([128, HL * D], BF, tag=f"wk{k}", name=f"wk{k}") for k in range(NKT)]
        wv_s = [persist.tile([128, HL * D], BF, tag=f"wv{k}", name=f"wv{k}") for k in range(NKT)]
        wo_s = [persist.tile([128, HID], BF, tag=f"wo{k}", name=f"wo{k}") for k in range(HL)]

        engs = [nc.sync, nc.scalar, nc.gpsimd]
        # phase 1: wq[k] + hT[k] first half (feeds the first Q waves).
        # k=0 is split across all three queues so the first wave's
        # k-chain can start as soon as possible after the warm-up.
        nc.sync.dma_start(out=wq_s[0], in_=wq[0:128, :])
        nc.scalar.dma_start(out=hT[0][:, 0:512], in_=ht[0:128, 0:512])
        nc.gpsimd.dma_start(out=hT[0][:, 512:1024], in_=ht[0:128, 512:1024])
        nc.scalar.dma_start(out=wq_s[1], in_=wq[128:256, :])
        nc.gpsimd.dma_start(out=hT[1][:, 0:512], in_=ht[128:256, 0:512])
        nc.sync.dma_start(out=hT[1][:, 512:1024], in_=ht[128:256, 512:1024])
        for k in range(2, NKT):
            ks = slice(128 * k, 128 * (k + 1))
            e = engs[k % 3]
            e.dma_start(out=wq_s[k], in_=wq[ks, :])
            e.dma_start(out=hT[k][:, 0:1024], in_=ht[ks, 0:1024])
        # phase 2: wk[k] + hT[k] second half
        for k in range(NKT):
            ks = slice(128 * k, 128 * (k + 1))
            e = engs[k % 3]
            e.dma_start(out=wk_s[k], in_=wk[ks, :])
            e.dma_start(out=hT[k][:, 1024:2048], in_=ht[ks, 1024:2048])
        # phase 3: wv + wo (consumed last)
        for k in range(NKT):
            ks = slice(128 * k, 128 * (k + 1))
            engs[1 + k % 2].dma_start(out=wv_s[k], in_=wv[ks, :])
        for k in range(HL):
            engs[1 + k % 2].dma_start(out=wo_s[k], in_=wo[128 * k : 128 * (k + 1), :])

        V = [persist.tile([128, HL * D], BF, tag=f"v{t}", name=f"v{t}") for t in range(NQ)]
        QT = [persist.tile([128, S], BF, tag=f"qt{h}", name=f"qt{h}") for h in range(HL)]
        KT = [persist.tile([128, S], BF, tag=f"kt{h}", name=f"kt{h}") for h in range(HL)]
        AO_T = [persist.tile([128, S], BF, tag=f"ao{hh}", name=f"ao{hh}") for hh in range(HL)]

        def qk_waves(w_tiles, dst, is_q, mcs, wave_sz=8, alt_copy=False):
            # (head, mc) chains for mc in mcs in waves (one PSUM bank
            # per chain): each wave walks k ONCE so an arriving
            # (w[k], hT[k]) pair feeds wave_sz*512 PE cycles.  Copies
            # stay on vector: scalar/gpsimd issue DMAs and can BLOCK on
            # a full DMA ring, which would stall the wave pipeline.
            chains = [(hh, mc) for hh in range(HL) for mc in mcs]
            for w0 in range(0, len(chains), wave_sz):
                wave = chains[w0 : w0 + wave_sz]
                ps = [ps_wave.tile([128, 512], F32, tag="big", name=f"wvps{i}")
                      for i in range(len(wave))]
                for k in range(NKT):
                    for i, (hh, mc) in enumerate(wave):
                        nc.tensor.matmul(
                            ps[i],
                            lhsT=w_tiles[k][:, 128 * hh : 128 * (hh + 1)],
                            rhs=hT[k][:, 512 * mc : 512 * (mc + 1)],
                            start=(k == 0), stop=(k == NKT - 1),
                        )
                for i, (hh, mc) in enumerate(wave):
                    # alt_copy halves the end-of-wave PSUM drain across
                    # vector+scalar; only safe once the scalar engine is
                    # done issuing input DMAs (it can block on a full
                    # DMA ring), i.e. for waves after the first
                    ms = slice(512 * mc, 512 * (mc + 1))
                    if alt_copy and i % 2 == 1:
                        nc.scalar.activation(
                            dst[hh][:, ms], ps[i],
                            mybir.ActivationFunctionType.Copy,
                            bias=0.0, scale=SCALE if is_q else 1.0,
                        )
                    elif is_q:
                        # fold the 1/sqrt(d) scaling into Q
                        nc.vector.tensor_scalar_mul(dst[hh][:, ms], ps[i], SCALE)
                    else:
                        nc.vector.tensor_copy(dst[hh][:, ms], ps[i])

        qk_waves(wq_s, QT, True, (0, 1))    # needs wq + hT first halves
        qk_waves(wk_s, KT, False, (0, 1), alt_copy=True)
        qk_waves(wq_s, QT, True, (2, 3), alt_copy=True)
        # data is resident by now; half-waves let the last PSUM drain
        # overlap the next half-wave's matmuls
        qk_waves(wk_s, KT, False, (2, 3), wave_sz=4, alt_copy=True)

        # release the wave pool's 8 banks, then lay out the V/attention
        # /Wo PSUM pools
        wave_ctx.close()
        # dedicated banks: V (1) and Wo (2) never contend with each
        # other; scores get 2 banks because exp reads straight from
        # PSUM so the bank is held until the exp completes (bufs=1
        # would stall the next tile's scores matmul in attention-dense
        # tail slots)
        ps_v = ctx.enter_context(tc.tile_pool(name="ps_v", bufs=1, space="PSUM"))
        ps_wo = ctx.enter_context(tc.tile_pool(name="ps_wo", bufs=2, space="PSUM"))
        ps_sc = ctx.enter_context(tc.tile_pool(name="ps_sc", bufs=2, space="PSUM"))
        ps_pt = ctx.enter_context(tc.tile_pool(name="ps_pt", bufs=2, space="PSUM"))
        ps_ao = ctx.enter_context(tc.tile_pool(name="ps_ao", bufs=1, space="PSUM"))

        def att_tile(hh, qt):
            hs_ = slice(128 * hh, 128 * (hh + 1))
            t0 = max(0, 128 * qt - 128)
            t1 = min(S, 128 * qt + 256)
            W = t1 - t0
            scps = ps_sc.tile([128, W], F32, tag="sc", name="scps")
            nc.tensor.matmul(
                scps, lhsT=QT[hh][:, 128 * qt : 128 * (qt + 1)],
                rhs=KT[hh][:, t0:t1],
                start=True, stop=True,
            )
            # corner masking straight in PSUM (vector engine memsets;
            # gpsimd cannot write PSUM) -- no mask tiles, no add pass
            if qt == 0:
                nc.vector.memset(scps[0:64, 192:256], NEG)
            elif qt == NQ - 1:
                nc.vector.memset(scps[64:128, 0:64], NEG)
            else:
                nc.vector.memset(scps[0:64, 320:384], NEG)
                nc.vector.memset(scps[64:128, 0:64], NEG)
            # scores are O(+-8) so exp needs no max subtraction
            # (softmax is shift-invariant; fp32 exp is safe here)
            p = work.tile([128, W], BF, tag="p", name="p")
            rsum = stats.tile([128, 1], F32, tag="rsum", name="rsum")
            nc.scalar.activation(
                p, scps, mybir.ActivationFunctionType.Exp,
                bias=0.0, scale=1.0, accum_out=rsum,
            )
            rcp = stats.tile([128, 1], F32, tag="rcp", name="rcp")
            nc.vector.reciprocal(rcp, rsum)
            nc.vector.tensor_scalar_mul(p, p, rcp)
            nch = W // 128
            aops = ps_ao.tile([128, 128], F32, tag="ao", name="aops")
            # ONE PSUM tile holds all this tile's transposed chunks
            # (bf16 [128,W] fits one bank): the chunks never contend
            # for pt buffers, so transpose ci never waits on an
            # earlier chunk's PSUM->SBUF copy
            ptps = ps_pt.tile([128, W], BF, tag="pt", name="ptps")
            for ci in range(nch):
                cs = slice(128 * ci, 128 * (ci + 1))
                nc.tensor.transpose(ptps[:, cs], p[:, cs], ident)
                pts = work.tile([128, 128], BF, tag="pts", name="pts")
                if ci % 2 == 1:
                    nc.vector.tensor_copy(pts, ptps[:, cs])
                else:
                    nc.scalar.copy(pts, ptps[:, cs])
                tt = t0 // 128 + ci
                nc.tensor.matmul(
                    aops, lhsT=V[tt][:, hs_], rhs=pts,
                    start=(ci == 0), stop=(ci == nch - 1),
                )
            # alternate the AO^T drain engine: the scalar engine gets
            # backed up in the tail slots and delays the Wo chains
            if hh % 2 == 0:
                nc.scalar.copy(AO_T[hh][:, 128 * qt : 128 * (qt + 1)], aops)
            else:
                nc.vector.tensor_copy(AO_T[hh][:, 128 * qt : 128 * (qt + 1)], aops)

        # ONE interleaved slot loop for everything after the QK waves:
        # slot t runs V_t, att0(t-1), att1(t-2), att2(t-3), att3(t-4)
        # and Wo(t-6).  Every att head h tile qt needs V up to qt+1
        # (satisfied by its lag), and every Wo(mt) needs AO of all four
        # heads at mt (AO3(mt) lands at slot mt+4, so lag 2).  The PE
        # stays dense through the whole stretch, and the final Wo group
        # depends on a softmax chain finished ~2 slots earlier.
        for t in range(NQ + 6):
            if t < NQ:
                vps = ps_v.tile([128, 512], F32, tag="v", name="vps")
                ts_ = slice(128 * t, 128 * (t + 1))
                for k in range(NKT):
                    nc.tensor.matmul(
                        vps, lhsT=hT[k][:, ts_], rhs=wv_s[k],
                        start=(k == 0), stop=(k == NKT - 1),
                    )
                nc.vector.tensor_copy(V[t], vps)
            for hh in range(HL):
                qt = t - 1 - hh
                if 0 <= qt < NQ:
                    att_tile(hh, qt)
            mt = t - 6
            if 0 <= mt < NQ - 1:
                if mt == NQ - 2:
                    _emit_wo(nc, ps_wo, osb_pool, AO_T, wo_s, out, mt,
                             out_engs=(nc.sync, nc.scalar), fine=True)
                else:
                    _emit_wo(nc, ps_wo, osb_pool, AO_T, wo_s, out, mt)
        # final group: spread the copies and output DMAs over both
        # engines/queues so the tail drains on two queues
        _emit_wo(nc, ps_wo, osb_pool, AO_T, wo_s, out, NQ - 1,
                 out_engs=(nc.sync, nc.scalar), fine=True)
        # keep the PE (and the package clock) busy while the final
        # output DMAs drain -- an idle PE demotes the DVFS state and
        # slows the very tail of the kernel
        tail_ps = ps_v.tile([128, 512], F32, tag="v", name="tail_ps")
        for _ in range(12):
            nc.tensor.matmul(tail_ps, lhsT=warm_sb, rhs=wo_s[0][:, 0:512],
                             start=True, stop=True)

    if not nc.is_finalized():
        nc.finalize()
    return nc


_NC = None


def _get_nc():
    global _NC
    if _NC is None:
        _NC = build()
    return _NC


def _in_maps(hidden_states, Wq, Wk, Wv, Wo):
    import ml_dtypes

    bf = ml_dtypes.bfloat16
    hs = np.asarray(hidden_states, dtype=np.float32)
    Wq = np.asarray(Wq, dtype=np.float32)
    Wk = np.asarray(Wk, dtype=np.float32)
    Wv = np.asarray(Wv, dtype=np.float32)
    Wo = np.asarray(Wo, dtype=np.float32)
    maps = []
    for c in range(8):
        b, g = divmod(c, 4)
        sl = slice(512 * g, 512 * (g + 1))
        maps.append(
            {
                "ht": np.ascontiguousarray(hs[b].T).astype(bf),
                "wq": np.ascontiguousarray(Wq[:, sl]).astype(bf),
                "wk": np.ascontiguousarray(Wk[:, sl]).astype(bf),
                "wv": np.ascontiguousarray(Wv[:, sl]).astype(bf),
                "wo": np.ascontiguousarray(Wo[sl, :]).astype(bf),
            }
        )
    return maps


def _gather(results):
    outs = [np.asarray(results[c]["out"]).astype(np.float32) for c in range(8)]
    return np.stack(
        [outs[0] + outs[1] + outs[2] + outs[3],
         outs[4] + outs[5] + outs[6] + outs[7]]
    )


def run(in_maps, trace=False, **kw):
    nc = _get_nc()
    return run_bass_kernel_spmd(nc, in_maps, core_ids=list(range(8)), trace=trace, **kw)


def kernel(hidden_states, Wq, Wk, Wv, Wo):
    maps = _in_maps(hidden_states, Wq, Wk, Wv, Wo)
    res = run(maps)
    return _gather(res.results)


# revision 31
# speedup vs baseline: 1.0198x; 1.0198x over previous
"""Block-sparse attention Trainium2 kernel (8 NeuronCores, SPMD).

Problem: hidden_states [2, 2048, 2048] fp32; Wq/Wk/Wv [2048, 2048]; Wo
[2048, 2048]. 16 heads x 128 dim, block-banded attention (BLOCK=64,
bandwidth 2 -> each 128-query tile attends a 384-key band with two
64x64 invalid corners).

Sharding: core c = (batch b = c//4) x (head group g = c%4, 4 heads).
Each core computes q/k/v projections for its 4 heads (columns of
Wq/Wk/Wv), banded attention, and a partial output through its rows of
Wo. Host sums the 4 partials per batch. No collectives.

Per-core schedule (all matmuls bf16, fp32 PSUM accumulate; inputs
pre-transposed/cast to bf16 host-side during sharding):
  1. A zeroed warm-up tile is memset first so 56 dependency-free
     matmuls ramp the PE DVFS clock before any DMA data arrives;
     the transpose identity is built after them.
  2. Loads are k-striped across the three DMA queues (sync/scalar/
     gpsimd) in consumption order: (wq[k], hT[k] first half) pairs
     (k=0 split across all three queues), then (wk[k], hT[k] second
     half), then wv/wo.
  3. Q/K projections run as k-synchronized 8-chain WAVES holding all
     8 PSUM banks (a scoped pool): every arriving (w[k], hT[k]) pair
     feeds 8*512 PE cycles, keeping the PE dense and its DVFS clock
     at the K=8 (2.4GHz) state.  QT/KT are produced directly
     transposed (lhsT=weight slice, rhs=hT) and persist for all 4
     heads.
  4. One interleaved slot loop runs everything else: slot t does the
     V-projection tile V_t, attention tiles att_h(t-1-h) for the four
     heads, and the output-projection group Wo(t-6).  Attention lags
     satisfy the V band (qt+1 <= t) and every Wo group's AO inputs
     finish two slots earlier, so the PE never waits on an in-flight
     softmax chain and no phase goes sparse (which would demote the
     PE clock).
  Attention tile: scores = QT^T KT band -> corner mask via two 64x64
  PSUM memsets (vector engine; no mask tiles, no mask-add pass) ->
  exp straight out of PSUM with fused rowsum accumulation (no max
  subtract; scores are O(+-8)) -> reciprocal -> normalize P ->
  interleaved PE-transpose / PV chunks (copies alternate scalar and
  vector) -> AO^T drain alternating scalar/vector by head.
  PSUM banks: V 1 / Wo 2 / scores 2 / transpose 2 / PV-accum 1.
  The scores pool needs 2 banks because the exp reads PSUM directly,
  holding its bank ~1.5us; with 1 bank the next tile's scores matmul
  stalls the PE in attention-dense tail slots and demotes the clock.
  Wo group: out[mt] = sum_h AO_h^T^T @ Wo rows, PSUM->SBUF copies
  alternating vector/scalar, output DMAs on the sync queue (the last
  groups split each copy AND its DMA across both engines/queues).
  After the final group, 12 throwaway matmuls keep the PE busy while
  the last output DMAs drain (an idle PE demotes the DVFS state).
Measured: ~283 us HW exec (core 0; ~80.8% bf16 MFU), rel err
~6.2e-3 vs the fp32 reference.  Rejected directions (measured
slower): XBAR DMA transpose of P (~1.2us issue cost per transfer on
the in-order HWDGE queues -> PE starves, clock demotes, 383us);
key-major S^T scores with gpsimd partition_all_reduce rowsums
(~1.6us per reduce, gpsimd saturates, 449us); grouping the three
transposes before the three PV matmuls (PSUM buffer contention,
+14us); single shared [128,W] transpose PSUM tile (coarser release
granularity, +6us); V pre-wave + one-slot-earlier attention lags
(+3us); interior output DMAs on the gpsimd software-DGE queue
(+2us).
"""

from contextlib import ExitStack

import numpy as np

import concourse.bass as bass
import concourse.mybir as mybir
import concourse.tile as tile
from concourse import bacc
from concourse.bass_utils import run_bass_kernel_spmd
from concourse.masks import make_identity

S = 2048          # sequence length
HID = 2048        # hidden size
HL = 4            # heads per core
D = 128           # head dim
NKT = HID // 128  # 16 contraction tiles
NQ = S // 128     # 16 query tiles
SCALE = float(D) ** -0.5
NEG = -1e30
BF = mybir.dt.bfloat16
F32 = mybir.dt.float32


def _emit_wo(nc, ps_wo, osb_pool, AO_T, wo_s, out, mt, out_engs=None, fine=False):
    mts = slice(128 * mt, 128 * (mt + 1))
    if out_engs is None:
        out_engs = (nc.sync,)
    for nc_ in range(4):
        ns = slice(512 * nc_, 512 * (nc_ + 1))
        ops_ = ps_wo.tile([128, 512], mybir.dt.float32, tag="wo", name="wops")
        for dk in range(HL):
            nc.tensor.matmul(
                ops_, lhsT=AO_T[dk][:, mts], rhs=wo_s[dk][:, ns],
                start=(dk == 0), stop=(dk == HL - 1),
            )
        osb = osb_pool.tile([128, 512], BF, tag="osb", name="osb")
        if fine:
            # tail: both engines drain half the tile in parallel, and
            # each half DMAs out on its own queue as soon as it lands
            nc.vector.tensor_copy(osb[:, 0:256], ops_[:, 0:256])
            nc.scalar.copy(osb[:, 256:512], ops_[:, 256:512])
            ns0 = slice(512 * nc_, 512 * nc_ + 256)
            ns1 = slice(512 * nc_ + 256, 512 * (nc_ + 1))
            e0 = out_engs[nc_ % len(out_engs)]
            e1 = out_engs[(nc_ + 1) % len(out_engs)]
            e0.dma_start(out=out[mts, ns0], in_=osb[:, 0:256])
            e1.dma_start(out=out[mts, ns1], in_=osb[:, 256:512])
        else:
            if nc_ % 2 == 0:
                nc.vector.tensor_copy(osb, ops_)
            else:
                nc.scalar.copy(osb, ops_)
            out_engs[nc_ % len(out_engs)].dma_start(out=out[mts, ns], in_=osb)


def build():
    nc = bacc.Bacc()
    # ht = h^T [hidden, seq]; all inputs pre-transposed/cast to bf16
    # host-side during sharding
    ht = nc.declare_dram_parameter("ht", [HID, S], BF, isOutput=False)
    wq = nc.declare_dram_parameter("wq", [HID, HL * D], BF, isOutput=False)
    wk = nc.declare_dram_parameter("wk", [HID, HL * D], BF, isOutput=False)
    wv = nc.declare_dram_parameter("wv", [HID, HL * D], BF, isOutput=False)
    wo = nc.declare_dram_parameter("wo", [HL * D, HID], BF, isOutput=False)
    out = nc.declare_dram_parameter("out", [S, HID], BF, isOutput=True)

    with ExitStack() as ctx:
        tc = ctx.enter_context(tile.TileContext(nc))
        persist = ctx.enter_context(tc.tile_pool(name="persist", bufs=1))
        work = ctx.enter_context(tc.tile_pool(name="work", bufs=4))
        stats = ctx.enter_context(tc.tile_pool(name="stats", bufs=8))
        osb_pool = ctx.enter_context(tc.tile_pool(name="osb", bufs=4))
        # the Q/K projection waves get ALL 8 PSUM banks via a scoped
        # pool; the attention/V/Wo pools are allocated after it closes
        wave_ctx = ExitStack()
        ps_wave = wave_ctx.enter_context(
            tc.tile_pool(name="ps_wave", bufs=8, space="PSUM"))

        # warm-up stationary: a zeroed tile whose memset is the very
        # first gpsimd op, so the PE ramps before any DMA data arrives
        warm_sb = persist.tile([128, 128], BF, tag="warm_sb")
        nc.gpsimd.memset(warm_sb, 0.0)
        warm_ps = ps_wave.tile([128, 512], F32, tag="big", name="warm_ps")
        for _ in range(56):
            nc.tensor.matmul(warm_ps[:, 0:128], lhsT=warm_sb, rhs=warm_sb,
                             start=True, stop=True)

        ident = persist.tile([128, 128], BF, tag="ident")
        make_identity(nc, ident)

        # ---- input loads (plain 2D DMAs, bf16), k-striped over the
        # three queues so arrival tracks the k-synchronized consumption
        # order; within each queue strictly priority-ordered.
        hT = [persist.tile([128, S], BF, tag=f"ht{k}", name=f"ht{k}") for k in range(NKT)]
        wq_s = [persist.tile([128, HL * D], BF, tag=f"wq{k}", name=f"wq{k}") for k in range(NKT)]
        wk_s = [persist.tile([128, HL * D], BF, tag=f"wk{k}", name=f"wk{k}") for k in range(NKT)]
        wv_s = [persist.tile([128, HL * D], BF, tag=f"wv{k}", name=f"wv{k}") for k in range(NKT)]
        wo_s = [persist.tile([128, HID], BF, tag=f"wo{k}", name=f"wo{k}") for k in range(HL)]

        engs = [nc.sync, nc.scalar, nc.gpsimd]
        # phase 1: wq[k] + hT[k] first half (feeds the first Q waves).
        # k=0 is split across all three queues so the first wave's
        # k-chain can start as soon as possible after the warm-up.
        nc.sync.dma_start(out=wq_s[0], in_=wq[0:128, :])
        nc.scalar.dma_start(out=hT[0][:, 0:512], in_=ht[0:128, 0:512])
        nc.gpsimd.dma_start(out=hT[0][:, 512:1024], in_=ht[0:128, 512:1024])
        for k in range(1, NKT):
            ks = slice(128 * k, 128 * (k + 1))
            e = engs[k % 3]
            e.dma_start(out=wq_s[k], in_=wq[ks, :])
            e.dma_start(out=hT[k][:, 0:1024], in_=ht[ks, 0:1024])
        # phase 2: wk[k] + hT[k] second half
        for k in range(NKT):
            ks = slice(128 * k, 128 * (k + 1))
            e = engs[k % 3]
            e.dma_start(out=wk_s[k], in_=wk[ks, :])
            e.dma_start(out=hT[k][:, 1024:2048], in_=ht[ks, 1024:2048])
        # phase 3: wv + wo (consumed last)
        for k in range(NKT):
            ks = slice(128 * k, 128 * (k + 1))
            engs[1 + k % 2].dma_start(out=wv_s[k], in_=wv[ks, :])
        for k in range(HL):
            engs[1 + k % 2].dma_start(out=wo_s[k], in_=wo[128 * k : 128 * (k + 1), :])

        V = [persist.tile([128, HL * D], BF, tag=f"v{t}", name=f"v{t}") for t in range(NQ)]
        QT = [persist.tile([128, S], BF, tag=f"qt{h}", name=f"qt{h}") for h in range(HL)]
        KT = [persist.tile([128, S], BF, tag=f"kt{h}", name=f"kt{h}") for h in range(HL)]
        AO_T = [persist.tile([128, S], BF, tag=f"ao{hh}", name=f"ao{hh}") for hh in range(HL)]

        def qk_waves(w_tiles, dst, is_q, mcs, wave_sz=8, alt_copy=False):
            # (head, mc) chains for mc in mcs in waves (one PSUM bank
            # per chain): each wave walks k ONCE so an arriving
            # (w[k], hT[k]) pair feeds wave_sz*512 PE cycles.  Copies
            # stay on vector: scalar/gpsimd issue DMAs and can BLOCK on
            # a full DMA ring, which would stall the wave pipeline.
            chains = [(hh, mc) for hh in range(HL) for mc in mcs]
            for w0 in range(0, len(chains), wave_sz):
                wave = chains[w0 : w0 + wave_sz]
                ps = [ps_wave.tile([128, 512], F32, tag="big", name=f"wvps{i}")
                      for i in range(len(wave))]
                for k in range(NKT):
                    for i, (hh, mc) in enumerate(wave):
                        nc.tensor.matmul(
                            ps[i],
                            lhsT=w_tiles[k][:, 128 * hh : 128 * (hh + 1)],
                            rhs=hT[k][:, 512 * mc : 512 * (mc + 1)],
                            start=(k == 0), stop=(k == NKT - 1),
                        )
                for i, (hh, mc) in enumerate(wave):
                    # alt_copy halves the end-of-wave PSUM drain across
                    # vector+scalar; only safe once the scalar engine is
                    # done issuing input DMAs (it can block on a full
                    # DMA ring), i.e. for waves after the first
                    ms = slice(512 * mc, 512 * (mc + 1))
                    if alt_copy and i % 2 == 1:
                        nc.scalar.activation(
                            dst[hh][:, ms], ps[i],
                            mybir.ActivationFunctionType.Copy,
                            bias=0.0, scale=SCALE if is_q else 1.0,
                        )
                    elif is_q:
                        # fold the 1/sqrt(d) scaling into Q
                        nc.vector.tensor_scalar_mul(dst[hh][:, ms], ps[i], SCALE)
                    else:
                        nc.vector.tensor_copy(dst[hh][:, ms], ps[i])

        qk_waves(wq_s, QT, True, (0, 1))    # needs wq + hT first halves
        qk_waves(wk_s, KT, False, (0, 1), alt_copy=True)
        qk_waves(wq_s, QT, True, (2, 3), alt_copy=True)
        # data is resident by now; half-waves let the last PSUM drain
        # overlap the next half-wave's matmuls
        qk_waves(wk_s, KT, False, (2, 3), wave_sz=4, alt_copy=True)

        # release the wave pool's 8 banks, then lay out the V/attention
        # /Wo PSUM pools
        wave_ctx.close()
        # dedicated banks: V (1) and Wo (2) never contend with each
        # other; scores get 2 banks because exp reads straight from
        # PSUM so the bank is held until the exp completes (bufs=1
        # would stall the next tile's scores matmul in attention-dense
        # tail slots)
        ps_v = ctx.enter_context(tc.tile_pool(name="ps_v", bufs=1, space="PSUM"))
        ps_wo = ctx.enter_context(tc.tile_pool(name="ps_wo", bufs=2, space="PSUM"))
        ps_sc = ctx.enter_context(tc.tile_pool(name="ps_sc", bufs=2, space="PSUM"))
        ps_pt = ctx.enter_context(tc.tile_pool(name="ps_pt", bufs=2, space="PSUM"))
        ps_ao = ctx.enter_context(tc.tile_pool(name="ps_ao", bufs=1, space="PSUM"))

        def att_tile(hh, qt):
            hs_ = slice(128 * hh, 128 * (hh + 1))
            t0 = max(0, 128 * qt - 128)
            t1 = min(S, 128 * qt + 256)
            W = t1 - t0
            scps = ps_sc.tile([128, W], F32, tag="sc", name="scps")
            nc.tensor.matmul(
                scps, lhsT=QT[hh][:, 128 * qt : 128 * (qt + 1)],
                rhs=KT[hh][:, t0:t1],
                start=True, stop=True,
            )
            # corner masking straight in PSUM (vector engine memsets;
            # gpsimd cannot write PSUM) -- no mask tiles, no add pass
            if qt == 0:
                nc.vector.memset(scps[0:64, 192:256], NEG)
            elif qt == NQ - 1:
                nc.vector.memset(scps[64:128, 0:64], NEG)
            else:
                nc.vector.memset(scps[0:64, 320:384], NEG)
                nc.vector.memset(scps[64:128, 0:64], NEG)
            # scores are O(+-8) so exp needs no max subtraction
            # (softmax is shift-invariant; fp32 exp is safe here)
            p = work.tile([128, W], BF, tag="p", name="p")
            rsum = stats.tile([128, 1], F32, tag="rsum", name="rsum")
            nc.scalar.activation(
                p, scps, mybir.ActivationFunctionType.Exp,
                bias=0.0, scale=1.0, accum_out=rsum,
            )
            rcp = stats.tile([128, 1], F32, tag="rcp", name="rcp")
            nc.vector.reciprocal(rcp, rsum)
            nc.vector.tensor_scalar_mul(p, p, rcp)
            nch = W // 128
            aops = ps_ao.tile([128, 128], F32, tag="ao", name="aops")
            for ci in range(nch):
                ptps = ps_pt.tile([128, 128], BF, tag="pt", name="ptps")
                nc.tensor.transpose(
                    ptps, p[:, 128 * ci : 128 * (ci + 1)], ident
                )
                pts = work.tile([128, 128], BF, tag="pts", name="pts")
                if ci % 2 == 1:
                    nc.vector.tensor_copy(pts, ptps)
                else:
                    nc.scalar.copy(pts, ptps)
                tt = t0 // 128 + ci
                nc.tensor.matmul(
                    aops, lhsT=V[tt][:, hs_], rhs=pts,
                    start=(ci == 0), stop=(ci == nch - 1),
                )
            # alternate the AO^T drain engine: the scalar engine gets
            # backed up in the tail slots and delays the Wo chains
            if hh % 2 == 0:
                nc.scalar.copy(AO_T[hh][:, 128 * qt : 128 * (qt + 1)], aops)
            else:
                nc.vector.tensor_copy(AO_T[hh][:, 128 * qt : 128 * (qt + 1)], aops)

        # ONE interleaved slot loop for everything after the QK waves:
        # slot t runs V_t, att0(t-1), att1(t-2), att2(t-3), att3(t-4)
        # and Wo(t-6).  Every att head h tile qt needs V up to qt+1
        # (satisfied by its lag), and every Wo(mt) needs AO of all four
        # heads at mt (AO3(mt) lands at slot mt+4, so lag 2).  The PE
        # stays dense through the whole stretch, and the final Wo group
        # depends on a softmax chain finished ~2 slots earlier.
        for t in range(NQ + 6):
            if t < NQ:
                vps = ps_v.tile([128, 512], F32, tag="v", name="vps")
                ts_ = slice(128 * t, 128 * (t + 1))
                for k in range(NKT):
                    nc.tensor.matmul(
                        vps, lhsT=hT[k][:, ts_], rhs=wv_s[k],
                        start=(k == 0), stop=(k == NKT - 1),
                    )
                nc.vector.tensor_copy(V[t], vps)
            for hh in range(HL):
                qt = t - 1 - hh
                if 0 <= qt < NQ:
                    att_tile(hh, qt)
            mt = t - 6
            if 0 <= mt < NQ - 1:
                if mt == NQ - 2:
                    _emit_wo(nc, ps_wo, osb_pool, AO_T, wo_s, out, mt,
                             out_engs=(nc.sync, nc.scalar), fine=True)
                else:
                    _emit_wo(nc, ps_wo, osb_pool, AO_T, wo_s, out, mt)
        # final group: spread the copies and output DMAs over both
        # engines/queues so the tail drains on two queues
        _emit_wo(nc, ps_wo, osb_pool, AO_T, wo_s, out, NQ - 1,
                 out_engs=(nc.sync, nc.scalar), fine=True)
        # keep the PE (and the package clock) busy while the final
        # output DMAs drain -- an idle PE demotes the DVFS state and
        # slows the very tail of the kernel
        tail_ps = ps_v.tile([128, 512], F32, tag="v", name="tail_ps")
        for _ in range(12):
            nc.tensor.matmul(tail_ps, lhsT=warm_sb, rhs=wo_s[0][:, 0:512],
                             start=True, stop=True)

    if not nc.is_finalized():
        nc.finalize()
    return nc


_NC = None


def _get_nc():
    global _NC
    if _NC is None:
        _NC = build()
    return _NC


def _in_maps(hidden_states, Wq, Wk, Wv, Wo):
    import ml_dtypes

    bf = ml_dtypes.bfloat16
    hs = np.asarray(hidden_states, dtype=np.float32)
    Wq = np.asarray(Wq, dtype=np.float32)
    Wk = np.asarray(Wk, dtype=np.float32)
    Wv = np.asarray(Wv, dtype=np.float32)
    Wo = np.asarray(Wo, dtype=np.float32)
    maps = []
    for c in range(8):
        b, g = divmod(c, 4)
        sl = slice(512 * g, 512 * (g + 1))
        maps.append(
            {
                "ht": np.ascontiguousarray(hs[b].T).astype(bf),
                "wq": np.ascontiguousarray(Wq[:, sl]).astype(bf),
                "wk": np.ascontiguousarray(Wk[:, sl]).astype(bf),
                "wv": np.ascontiguousarray(Wv[:, sl]).astype(bf),
                "wo": np.ascontiguousarray(Wo[sl, :]).astype(bf),
            }
        )
    return maps


def _gather(results):
    outs = [np.asarray(results[c]["out"]).astype(np.float32) for c in range(8)]
    return np.stack(
        [outs[0] + outs[1] + outs[2] + outs[3],
         outs[4] + outs[5] + outs[6] + outs[7]]
    )


def run(in_maps, trace=False, **kw):
    nc = _get_nc()
    return run_bass_kernel_spmd(nc, in_maps, core_ids=list(range(8)), trace=trace, **kw)


def kernel(hidden_states, Wq, Wk, Wv, Wo):
    maps = _in_maps(hidden_states, Wq, Wk, Wv, Wo)
    res = run(maps)
    return _gather(res.results)
